# revision 26
# baseline (speedup 1.0000x reference)
"""Trainium2 Bass kernel for the CNF-with-exact-Jacobian-trace problem.

Reference computation (B=2048, D=64, H=512):
    inp = [z, t]                      # time-augmented input, [D+1]
    h1  = tanh(inp @ W1 + b1)         # [H]
    h2  = tanh(h1 @ W2 + b2)          # [H]
    dz  = h2 @ W3 + b3                # [D]
    J   = d(dz)/dz                    # [D, D] per sample
    dlogp = -trace(J)

Key algebraic identity (avoids materializing J entirely):
    trace(J)_b = d1_b^T (W2 * (W3 @ W1z)^T) d2_b
where d1 = 1-h1^2, d2 = 1-h2^2, W1z = W1[:D], and * is elementwise.
So per sample the trace is a bilinear form through the H x H matrix
C = W2 * M^T with M = W3 @ W1z (computed on-device from the weights).

Distribution: pure data-parallel over B across 8 NeuronCores (256
samples each); weights replicated. Hidden-layer matmuls run
feature-on-partition ("transposed" activations) so biases are
per-partition ACT bias vectors; the final layer runs back in natural
layout so dz DMAs out contiguously.

Host-side work is layout-only (transpose/reshape/concat/replicate and
optional dtype cast of inputs) - no arithmetic happens on the host.
"""

import sys

sys.path.insert(0, "/opt/trn_rl_repo")

import numpy as np

import concourse.bacc as bacc
import concourse.mybir as mybir
from concourse import tile
from concourse.bass_utils import run_bass_kernel_spmd

B, D, H = 2048, 64, 512
NCORES = 8
BS = B // NCORES          # 256 samples per core
KA = D + 2                # 66: z rows + t row + ones row
HC = H // 128             # 4 chunks of the hidden dim
BC = BS // 128            # 2 sample chunks of 128

F32 = mybir.dt.float32
AF = mybir.ActivationFunctionType
ALU = mybir.AluOpType

# matmul operand dtype: "bf16" | "fp32" | "fp32r"
MM_DTYPE = "bf16"

_CACHED = {}
_RUN_KWARGS = {}  # test harness may set {"trace": True} for profiling


def build_nc(mm_dtype=None):
    mm_dtype = mm_dtype or MM_DTYPE
    MD = mybir.dt.bfloat16 if mm_dtype == "bf16" else F32
    use_r = mm_dtype == "fp32r"

    def mm_ap(ap):
        """AP passed to matmul: optionally reinterpret f32 as float32r."""
        return ap.bitcast(mybir.dt.float32r) if use_r else ap

    nc = bacc.Bacc("TRN2", target_bir_lowering=False, debug=False, num_devices=NCORES)

    zaug = nc.declare_dram_parameter("zaug", [KA, BS], MD, isOutput=False)
    w1b = nc.declare_dram_parameter("w1b", [KA, H], MD, isOutput=False)
    w2ca = nc.declare_dram_parameter("w2ca", [128, 2, H], MD, isOutput=False)
    w2cb = nc.declare_dram_parameter("w2cb", [128, 2, H], MD, isOutput=False)
    w3c = nc.declare_dram_parameter("w3c", [128, HC, D], MD, isOutput=False)
    w3t = nc.declare_dram_parameter("w3t", [D, H], MD, isOutput=False)
    b2cc = nc.declare_dram_parameter("b2c", [128, HC], F32, isOutput=False)
    b3w = nc.declare_dram_parameter("b3w", [1, D], MD, isOutput=False)
    dz_out = nc.declare_dram_parameter("dz", [BS, D], F32, isOutput=True)
    dlp_out = nc.declare_dram_parameter("dlp", [BS, 1], F32, isOutput=True)

    with tile.TileContext(nc) as tc:
        with (
            tc.tile_pool(name="w", bufs=1) as wp,
            tc.tile_pool(name="act", bufs=1) as ap,
            tc.tile_pool(name="psm", bufs=2, space="PSUM") as pm,
            tc.tile_pool(name="psa", bufs=4, space="PSUM") as pa,
            tc.tile_pool(name="pso", bufs=1, space="PSUM") as po,
            tc.tile_pool(name="pst", bufs=1, space="PSUM") as pt,
        ):
            # ---- loads: per-queue FIFO latency is ~3us per DMA, so order
            # each queue by criticality: A1 deps first on the two HWDGE
            # queues, w3t on gpsimd, then the rest ----
            zaug_t = wp.tile([KA, BS], MD)
            nc.sync.dma_start(zaug_t[:], zaug[:])
            w1b_t = wp.tile([KA, H], MD)
            nc.scalar.dma_start(w1b_t[:], w1b[:])
            w3t_tt = wp.tile([D, H], MD)
            nc.gpsimd.dma_start(w3t_tt[:], w3t[:])
            w2ca_t = wp.tile([128, 2, H], MD)
            nc.sync.dma_start(w2ca_t[:], w2ca[:])
            w2cb_t = wp.tile([128, 2, H], MD)
            nc.scalar.dma_start(w2cb_t[:], w2cb[:])
            b2c_t = wp.tile([128, HC], F32)
            nc.gpsimd.dma_start(b2c_t[:], b2cc[:])
            w3c_t = wp.tile([128, HC, D], MD)
            nc.gpsimd.dma_start(w3c_t[:], w3c[:])
            b3w_t = wp.tile([1, D], MD)
            nc.sync.dma_start(b3w_t[:], b3w[:])

            w3t_t = w3t_tt[:]

            def w2_blk(pc, qc):
                t = w2ca_t if pc < 2 else w2cb_t
                return t[:, pc % 2, qc * 128 : (qc + 1) * 128]

            def w2_row(pc):
                t = w2ca_t if pc < 2 else w2cb_t
                return t[:, pc % 2, :]

            def w3_blk(qc):
                return w3c_t[:, qc, :]

            # trace-reduction vector is -1 so the ones-matmul computes
            # -sum_q E directly (= dlogp, no separate negate op)
            ones_t = wp.tile([128, 1], MD)
            nc.vector.memset(ones_t[:], -1.0)
            onesr_t = wp.tile([1, 128], MD)
            nc.vector.memset(onesr_t[:], 1.0)

            # ---- layer 1: A1T = [W1; W1t; b1]^T @ [zT; t; 1]  (K=66) ----
            h1 = ap.tile([128, HC, BS], MD)
            ps_a1 = []
            for hc in range(HC):
                ps = pa.tile([128, BS], F32, tag="psa")
                nc.tensor.matmul(
                    ps[:],
                    mm_ap(w1b_t[:, hc * 128 : (hc + 1) * 128]),
                    mm_ap(zaug_t[:]),
                    start=True,
                    stop=True,
                )
                ps_a1.append(ps)
            sq = ap.tile([128, HC, BS], F32)
            d1 = ap.tile([128, HC, BS], MD)
            for hc in range(HC):
                nc.scalar.activation(h1[:, hc, :], ps_a1[hc][:], AF.Tanh)
                nc.vector.tensor_mul(sq[:, hc, :], h1[:, hc, :], h1[:, hc, :])
                nc.vector.tensor_scalar(
                    d1[:, hc, :], sq[:, hc, :], -1.0, 1.0, ALU.mult, ALU.add
                )

            # ---- C = W2 * (W1z^T @ W3^T): the Mt matmuls need only w3t +
            # w1b, so they run inside the DMA-wait bubble before A1 ----
            csb = wp.tile([128, HC, H], MD)
            for pc in range(HC):
                ps_m = pm.tile([128, H], F32, tag="psm")
                nc.tensor.matmul(
                    ps_m[:],
                    mm_ap(w1b_t[0:D, pc * 128 : (pc + 1) * 128]),
                    mm_ap(w3t_t[:]),
                    start=True,
                    stop=True,
                )
                nc.vector.tensor_mul(csb[:, pc, :], ps_m[:], w2_row(pc))

            # ---- layer 2: A2T = W2^T @ h1T + b2 ----
            h2 = ap.tile([128, HC, BS], MD)
            d2 = ap.tile([128, HC, BS], F32)
            for qc in range(HC):
                ps_a2 = pa.tile([128, BS], F32, tag="psa")
                for pc in range(HC):
                    nc.tensor.matmul(
                        ps_a2[:],
                        mm_ap(w2_blk(pc, qc)),
                        mm_ap(h1[:, pc, :]),
                        start=(pc == 0),
                        stop=(pc == HC - 1),
                    )
                nc.scalar.activation(
                    h2[:, qc, :], ps_a2[:], AF.Tanh, bias=b2c_t[:, qc : qc + 1]
                )
                nc.vector.tensor_mul(sq[:, qc, :], h2[:, qc, :], h2[:, qc, :])
                nc.vector.tensor_scalar(
                    d2[:, qc, :], sq[:, qc, :], -1.0, 1.0, ALU.mult, ALU.add
                )

            # ---- layer 3 (natural layout): dz = h2 @ W3 + b3 ----
            # Emitted before the trace path so the dz DMA receipt overlaps
            # the u/tr matmuls. b3 is folded in as a K=1 matmul (ones_row^T
            # @ b3row) and the PSUM eviction runs on the idle ACT engine,
            # keeping the DVE free for the E = u * d2 multiplies.
            dz_sb = ap.tile([128, BC, D], F32)
            for bc in range(BC):
                ps_o = po.tile([128, D], F32, tag="pso")
                for qc in range(HC):
                    nc.tensor.matmul(
                        ps_o[:],
                        mm_ap(h2[:, qc, bc * 128 : (bc + 1) * 128]),
                        mm_ap(w3_blk(qc)),
                        start=(qc == 0),
                        stop=False,
                    )
                nc.tensor.matmul(
                    ps_o[:], mm_ap(onesr_t[:]), mm_ap(b3w_t[:]),
                    start=False, stop=True,
                )
                nc.scalar.copy(dz_sb[:, bc, :], ps_o[:])
            nc.sync.dma_start(
                dz_out.rearrange("(n p) j -> p n j", p=128), dz_sb[:]
            )

            # ---- u = C^T @ d1, E = u * d2, and the trace reduction
            # interleaved: tr_{q-1} rides between u-groups so only the last
            # trace matmul waits on its E-multiply ----
            esb = ap.tile([128, HC, BS], MD)
            ps_tr = pt.tile([1, BS], F32)
            for qc in range(HC):
                ps_u = pa.tile([128, BS], F32, tag="psa")
                for pc in range(HC):
                    nc.tensor.matmul(
                        ps_u[:],
                        mm_ap(csb[:, pc, qc * 128 : (qc + 1) * 128]),
                        mm_ap(d1[:, pc, :]),
                        start=(pc == 0),
                        stop=(pc == HC - 1),
                    )
                nc.vector.tensor_mul(esb[:, qc, :], ps_u[:], d2[:, qc, :])
                if qc >= 1:
                    nc.tensor.matmul(
                        ps_tr[:],
                        mm_ap(ones_t[:]),
                        mm_ap(esb[:, qc - 1, :]),
                        start=(qc - 1 == 0),
                        stop=False,
                    )

            # ---- dlogp: close the trace accumulation (ones vector is -1,
            # so psT = -sum_q E = dlogp directly) ----
            nc.tensor.matmul(
                ps_tr[:],
                mm_ap(ones_t[:]),
                mm_ap(esb[:, HC - 1, :]),
                start=False,
                stop=True,
            )
            tr_sb = ap.tile([1, BS], F32)
            nc.vector.tensor_copy(tr_sb[:], ps_tr[:])
            nc.scalar.dma_start(dlp_out.rearrange("b o -> o b"), tr_sb[:])

    nc.compile()
    return nc


def _np_md(mm_dtype):
    if mm_dtype == "bf16":
        import ml_dtypes

        return ml_dtypes.bfloat16
    return np.float32


def _prep_shared(t, W1, b1, W2, b2, W3, b3, mm_dtype):
    """Host-side layout prep of the replicated weight tensors."""
    md = _np_md(mm_dtype)
    w1b = np.concatenate([W1, b1[None, :]], axis=0).astype(md)      # [66, 512]
    w2all = np.ascontiguousarray(
        W2.reshape(HC, 128, H).transpose(1, 0, 2)).astype(md)
    w2ca, w2cb = np.ascontiguousarray(w2all[:, :2]), np.ascontiguousarray(w2all[:, 2:])
    w3c = np.ascontiguousarray(
        W3.reshape(HC, 128, D).transpose(1, 0, 2)).astype(md)
    w3t = np.ascontiguousarray(W3.T).astype(md)                     # [64, 512]
    b2c = np.ascontiguousarray(b2.reshape(HC, 128).T)               # [128, 4]
    b3w = np.ascontiguousarray(b3[None, :]).astype(md)              # [1, 64]
    return dict(w1b=w1b, w2ca=w2ca, w2cb=w2cb, w3c=w3c, w3t=w3t, b2c=b2c, b3w=b3w)


def kernel(z, logp_z, t, W1, b1, W2, b2, W3, b3):
    z = np.asarray(z, np.float32)
    t = np.asarray(t, np.float32)
    W1 = np.asarray(W1, np.float32)
    b1 = np.asarray(b1, np.float32)
    W2 = np.asarray(W2, np.float32)
    b2 = np.asarray(b2, np.float32)
    W3 = np.asarray(W3, np.float32)
    b3 = np.asarray(b3, np.float32)

    builder = _CACHED.get("builder", build_nc)
    key = ("nc", MM_DTYPE, builder.__name__)
    if key not in _CACHED:
        _CACHED[key] = builder(MM_DTYPE)
    nc = _CACHED[key]

    md = _np_md(MM_DTYPE)
    shared = _prep_shared(t, W1, b1, W2, b2, W3, b3, MM_DTYPE)
    in_maps = []
    for c in range(NCORES):
        zs = z[c * BS : (c + 1) * BS]                          # [256, 64]
        zaug = np.empty((KA, BS), np.float32)
        zaug[:D] = zs.T
        zaug[D] = t[0]
        zaug[D + 1] = 1.0
        in_maps.append({"zaug": zaug.astype(md), **shared})

    res = run_bass_kernel_spmd(nc, in_maps, list(range(NCORES)), **_RUN_KWARGS)
    _CACHED["last_results"] = res
    dz = np.concatenate([r["dz"] for r in res.results], axis=0)
    dlp = np.concatenate([r["dlp"] for r in res.results], axis=0)
    return dz, dlp


if __name__ == "__main__":
    rng = np.random.default_rng(0)
    inputs = {
        "z": rng.standard_normal((B, D)).astype(np.float32),
        "logp_z": np.zeros((B, 1), np.float32),
        "t": rng.random((1,)).astype(np.float32),
        "W1": (rng.standard_normal((D + 1, H)) / np.sqrt(D + 1)).astype(np.float32),
        "b1": np.zeros((H,), np.float32),
        "W2": (rng.standard_normal((H, H)) / np.sqrt(H)).astype(np.float32),
        "b2": np.zeros((H,), np.float32),
        "W3": (rng.standard_normal((H, D)) / np.sqrt(H)).astype(np.float32),
        "b3": np.zeros((D,), np.float32),
    }
    dz, dlp = kernel(**inputs)
    print(dz.shape, dlp.shape, dz.dtype, dlp.dtype)


# revision 27
# speedup vs baseline: 1.0774x; 1.0774x over previous
"""Trainium2 Bass kernel for the CNF-with-exact-Jacobian-trace problem.

Reference computation (B=2048, D=64, H=512):
    inp = [z, t]                      # time-augmented input, [D+1]
    h1  = tanh(inp @ W1 + b1)         # [H]
    h2  = tanh(h1 @ W2 + b2)          # [H]
    dz  = h2 @ W3 + b3                # [D]
    J   = d(dz)/dz                    # [D, D] per sample
    dlogp = -trace(J)

Key algebraic identity (avoids materializing J entirely):
    trace(J)_b = d1_b^T (W2 * (W3 @ W1z)^T) d2_b
where d1 = 1-h1^2, d2 = 1-h2^2, W1z = W1[:D], and * is elementwise.
So per sample the trace is a bilinear form through the H x H matrix
C = W2 * M^T with M = W3 @ W1z (computed on-device from the weights).

Distribution: pure data-parallel over B across 8 NeuronCores (256
samples each); weights replicated. Hidden-layer matmuls run
feature-on-partition ("transposed" activations) so biases are
per-partition ACT bias vectors; the final layer runs back in natural
layout so dz DMAs out contiguously.

Host-side work is layout-only (transpose/reshape/concat/replicate and
optional dtype cast of inputs) - no arithmetic happens on the host.
"""

import sys

sys.path.insert(0, "/opt/trn_rl_repo")

import numpy as np

import concourse.bacc as bacc
import concourse.mybir as mybir
from concourse import tile
from concourse.bass_utils import run_bass_kernel_spmd

B, D, H = 2048, 64, 512
NCORES = 8
BS = B // NCORES          # 256 samples per core
KA = D + 2                # 66: z rows + t row + ones row
HC = H // 128             # 4 chunks of the hidden dim
BC = BS // 128            # 2 sample chunks of 128

F32 = mybir.dt.float32
AF = mybir.ActivationFunctionType
ALU = mybir.AluOpType

# matmul operand dtype: "bf16" | "fp32" | "fp32r"
MM_DTYPE = "bf16"

_CACHED = {}
_RUN_KWARGS = {}  # test harness may set {"trace": True} for profiling


def build_nc(mm_dtype=None):
    mm_dtype = mm_dtype or MM_DTYPE
    MD = mybir.dt.bfloat16 if mm_dtype == "bf16" else F32
    use_r = mm_dtype == "fp32r"

    def mm_ap(ap):
        """AP passed to matmul: optionally reinterpret f32 as float32r."""
        return ap.bitcast(mybir.dt.float32r) if use_r else ap

    nc = bacc.Bacc("TRN2", target_bir_lowering=False, debug=False, num_devices=NCORES)

    zaug = nc.declare_dram_parameter("zaug", [KA, BS], MD, isOutput=False)
    w1b = nc.declare_dram_parameter("w1b", [KA, H], MD, isOutput=False)
    w2ca = nc.declare_dram_parameter("w2ca", [128, 2, H], MD, isOutput=False)
    w2cb = nc.declare_dram_parameter("w2cb", [128, 2, H], MD, isOutput=False)
    w3c = nc.declare_dram_parameter("w3c", [128, HC, D], MD, isOutput=False)
    w3t = nc.declare_dram_parameter("w3t", [D, H], MD, isOutput=False)
    b2cc = nc.declare_dram_parameter("b2c", [128, HC], F32, isOutput=False)
    b3w = nc.declare_dram_parameter("b3w", [1, D], MD, isOutput=False)
    dz_out = nc.declare_dram_parameter("dz", [BS, D], F32, isOutput=True)
    dlp_out = nc.declare_dram_parameter("dlp", [BS, 1], F32, isOutput=True)

    with tile.TileContext(nc) as tc:
        with (
            tc.tile_pool(name="w", bufs=1) as wp,
            tc.tile_pool(name="act", bufs=1) as ap,
            tc.tile_pool(name="psm", bufs=2, space="PSUM") as pm,
            tc.tile_pool(name="psa", bufs=4, space="PSUM") as pa,
            tc.tile_pool(name="pso", bufs=1, space="PSUM") as po,
            tc.tile_pool(name="pst", bufs=1, space="PSUM") as pt,
        ):
            # ---- loads: per-queue FIFO latency is ~3us per DMA, so order
            # each queue by criticality: A1 deps first on the two HWDGE
            # queues, w3t on gpsimd, then the rest ----
            zaug_t = wp.tile([KA, BS], MD)
            nc.sync.dma_start(zaug_t[:], zaug[:])
            w1b_t = wp.tile([KA, H], MD)
            nc.scalar.dma_start(w1b_t[:], w1b[:])
            w3t_tt = wp.tile([D, H], MD)
            nc.gpsimd.dma_start(w3t_tt[:], w3t[:])
            w2ca_t = wp.tile([128, 2, H], MD)
            nc.sync.dma_start(w2ca_t[:], w2ca[:])
            w2cb_t = wp.tile([128, 2, H], MD)
            nc.scalar.dma_start(w2cb_t[:], w2cb[:])
            b2c_t = wp.tile([128, HC], F32)
            nc.gpsimd.dma_start(b2c_t[:], b2cc[:])
            w3c_t = wp.tile([128, HC, D], MD)
            nc.gpsimd.dma_start(w3c_t[:], w3c[:])
            b3w_t = wp.tile([1, D], MD)
            nc.sync.dma_start(b3w_t[:], b3w[:])

            w3t_t = w3t_tt[:]

            def w2_blk(pc, qc):
                t = w2ca_t if pc < 2 else w2cb_t
                return t[:, pc % 2, qc * 128 : (qc + 1) * 128]

            def w2_row(pc):
                t = w2ca_t if pc < 2 else w2cb_t
                return t[:, pc % 2, :]

            def w3_blk(qc):
                return w3c_t[:, qc, :]

            # trace-reduction vector is -1 so the ones-matmul computes
            # -sum_q E directly (= dlogp, no separate negate op)
            ones_t = wp.tile([128, 1], MD)
            nc.vector.memset(ones_t[:], -1.0)
            onesr_t = wp.tile([1, 128], MD)
            nc.vector.memset(onesr_t[:], 1.0)

            # ---- layer 1: A1T = [W1; W1t; b1]^T @ [zT; t; 1]  (K=66) ----
            h1 = ap.tile([128, HC, BS], MD)
            ps_a1 = []
            for hc in range(HC):
                ps = pa.tile([128, BS], F32, tag="psa")
                nc.tensor.matmul(
                    ps[:],
                    mm_ap(w1b_t[:, hc * 128 : (hc + 1) * 128]),
                    mm_ap(zaug_t[:]),
                    start=True,
                    stop=True,
                )
                ps_a1.append(ps)
            sq = ap.tile([128, HC, BS], F32)
            d1 = ap.tile([128, HC, BS], MD)
            for hc in range(HC):
                nc.scalar.activation(h1[:, hc, :], ps_a1[hc][:], AF.Tanh)
                nc.vector.tensor_mul(sq[:, hc, :], h1[:, hc, :], h1[:, hc, :])
                nc.vector.tensor_scalar(
                    d1[:, hc, :], sq[:, hc, :], -1.0, 1.0, ALU.mult, ALU.add
                )

            # ---- C = W2 * (W1z^T @ W3^T): the Mt matmuls need only w3t +
            # w1b, so they run inside the DMA-wait bubble before A1 ----
            csb = wp.tile([128, HC, H], MD)
            for pc in range(HC):
                ps_m = pm.tile([128, H], F32, tag="psm")
                nc.tensor.matmul(
                    ps_m[:],
                    mm_ap(w1b_t[0:D, pc * 128 : (pc + 1) * 128]),
                    mm_ap(w3t_t[:]),
                    start=True,
                    stop=True,
                )
                nc.vector.tensor_mul(csb[:, pc, :], ps_m[:], w2_row(pc))

            # ---- layer 2: A2T = W2^T @ h1T + b2 ----
            h2 = ap.tile([128, HC, BS], MD)
            d2 = ap.tile([128, HC, BS], F32)
            for qc in range(HC):
                ps_a2 = pa.tile([128, BS], F32, tag="psa")
                for pc in range(HC):
                    nc.tensor.matmul(
                        ps_a2[:],
                        mm_ap(w2_blk(pc, qc)),
                        mm_ap(h1[:, pc, :]),
                        start=(pc == 0),
                        stop=(pc == HC - 1),
                    )
                nc.scalar.activation(
                    h2[:, qc, :], ps_a2[:], AF.Tanh, bias=b2c_t[:, qc : qc + 1]
                )
                nc.vector.tensor_mul(sq[:, qc, :], h2[:, qc, :], h2[:, qc, :])
                nc.vector.tensor_scalar(
                    d2[:, qc, :], sq[:, qc, :], -1.0, 1.0, ALU.mult, ALU.add
                )

            # ---- u = C^T @ d1, E = u * d2, and the trace reduction
            # interleaved: tr_{q-1} rides between u-groups so only the last
            # trace matmul waits on its E-multiply ----
            esb = ap.tile([128, HC, BS], MD)
            ps_tr = pt.tile([1, BS], F32)
            for qc in range(HC):
                ps_u = pa.tile([128, BS], F32, tag="psa")
                for pc in range(HC):
                    nc.tensor.matmul(
                        ps_u[:],
                        mm_ap(csb[:, pc, qc * 128 : (qc + 1) * 128]),
                        mm_ap(d1[:, pc, :]),
                        start=(pc == 0),
                        stop=(pc == HC - 1),
                    )
                nc.vector.tensor_mul(esb[:, qc, :], ps_u[:], d2[:, qc, :])
                if qc >= 1:
                    nc.tensor.matmul(
                        ps_tr[:],
                        mm_ap(ones_t[:]),
                        mm_ap(esb[:, qc - 1, :]),
                        start=(qc - 1 == 0),
                        stop=False,
                    )

            # ---- layer 3 (natural layout): dz = h2 @ W3 + b3 ----
            # Emitted before the trace path so the dz DMA receipt overlaps
            # the u/tr matmuls. b3 is folded in as a K=1 matmul (ones_row^T
            # @ b3row) and the PSUM eviction runs on the idle ACT engine,
            # keeping the DVE free for the E = u * d2 multiplies.
            dz_sb = ap.tile([128, BC, D], F32)
            for bc in range(BC):
                ps_o = po.tile([128, D], F32, tag="pso")
                for qc in range(HC):
                    nc.tensor.matmul(
                        ps_o[:],
                        mm_ap(h2[:, qc, bc * 128 : (bc + 1) * 128]),
                        mm_ap(w3_blk(qc)),
                        start=(qc == 0),
                        stop=False,
                    )
                nc.tensor.matmul(
                    ps_o[:], mm_ap(onesr_t[:]), mm_ap(b3w_t[:]),
                    start=False, stop=True,
                )
                nc.scalar.copy(dz_sb[:, bc, :], ps_o[:])
            nc.sync.dma_start(
                dz_out.rearrange("(n p) j -> p n j", p=128), dz_sb[:]
            )

            # ---- dlogp: close the trace accumulation (ones vector is -1,
            # so psT = -sum_q E = dlogp directly) ----
            nc.tensor.matmul(
                ps_tr[:],
                mm_ap(ones_t[:]),
                mm_ap(esb[:, HC - 1, :]),
                start=False,
                stop=True,
            )
            tr_sb = ap.tile([1, BS], F32)
            nc.vector.tensor_copy(tr_sb[:], ps_tr[:])
            nc.scalar.dma_start(dlp_out.rearrange("b o -> o b"), tr_sb[:])

    nc.compile()
    return nc


def _np_md(mm_dtype):
    if mm_dtype == "bf16":
        import ml_dtypes

        return ml_dtypes.bfloat16
    return np.float32


def _prep_shared(t, W1, b1, W2, b2, W3, b3, mm_dtype):
    """Host-side layout prep of the replicated weight tensors."""
    md = _np_md(mm_dtype)
    w1b = np.concatenate([W1, b1[None, :]], axis=0).astype(md)      # [66, 512]
    w2all = np.ascontiguousarray(
        W2.reshape(HC, 128, H).transpose(1, 0, 2)).astype(md)
    w2ca, w2cb = np.ascontiguousarray(w2all[:, :2]), np.ascontiguousarray(w2all[:, 2:])
    w3c = np.ascontiguousarray(
        W3.reshape(HC, 128, D).transpose(1, 0, 2)).astype(md)
    w3t = np.ascontiguousarray(W3.T).astype(md)                     # [64, 512]
    b2c = np.ascontiguousarray(b2.reshape(HC, 128).T)               # [128, 4]
    b3w = np.ascontiguousarray(b3[None, :]).astype(md)              # [1, 64]
    return dict(w1b=w1b, w2ca=w2ca, w2cb=w2cb, w3c=w3c, w3t=w3t, b2c=b2c, b3w=b3w)


def kernel(z, logp_z, t, W1, b1, W2, b2, W3, b3):
    z = np.asarray(z, np.float32)
    t = np.asarray(t, np.float32)
    W1 = np.asarray(W1, np.float32)
    b1 = np.asarray(b1, np.float32)
    W2 = np.asarray(W2, np.float32)
    b2 = np.asarray(b2, np.float32)
    W3 = np.asarray(W3, np.float32)
    b3 = np.asarray(b3, np.float32)

    builder = _CACHED.get("builder", build_nc)
    key = ("nc", MM_DTYPE, builder.__name__)
    if key not in _CACHED:
        _CACHED[key] = builder(MM_DTYPE)
    nc = _CACHED[key]

    md = _np_md(MM_DTYPE)
    shared = _prep_shared(t, W1, b1, W2, b2, W3, b3, MM_DTYPE)
    in_maps = []
    for c in range(NCORES):
        zs = z[c * BS : (c + 1) * BS]                          # [256, 64]
        zaug = np.empty((KA, BS), np.float32)
        zaug[:D] = zs.T
        zaug[D] = t[0]
        zaug[D + 1] = 1.0
        in_maps.append({"zaug": zaug.astype(md), **shared})

    res = run_bass_kernel_spmd(nc, in_maps, list(range(NCORES)), **_RUN_KWARGS)
    _CACHED["last_results"] = res
    dz = np.concatenate([r["dz"] for r in res.results], axis=0)
    dlp = np.concatenate([r["dlp"] for r in res.results], axis=0)
    return dz, dlp


if __name__ == "__main__":
    rng = np.random.default_rng(0)
    inputs = {
        "z": rng.standard_normal((B, D)).astype(np.float32),
        "logp_z": np.zeros((B, 1), np.float32),
        "t": rng.random((1,)).astype(np.float32),
        "W1": (rng.standard_normal((D + 1, H)) / np.sqrt(D + 1)).astype(np.float32),
        "b1": np.zeros((H,), np.float32),
        "W2": (rng.standard_normal((H, H)) / np.sqrt(H)).astype(np.float32),
        "b2": np.zeros((H,), np.float32),
        "W3": (rng.standard_normal((H, D)) / np.sqrt(H)).astype(np.float32),
        "b3": np.zeros((D,), np.float32),
    }
    dz, dlp = kernel(**inputs)
    print(dz.shape, dlp.shape, dz.dtype, dlp.dtype)
